# revision 18
# baseline (speedup 1.0000x reference)
"""Trainium2 kernel for nn_Eq2Net_7859790151696.

Device (8 NeuronCores, SPMD, t-sharded): the head projections
logits = s_i @ [W_action | W_stop | W_start]  -- all of the input memory
traffic (s_i is 4.2 MB of the 4.85 MB total) and virtually all FLOPs.
Each core computes a 257-row t-shard of the (2049, 336) logits.

Host: the strictly-sequential T=2048, B=16 HMM recurrence, reformulated as a
chunked linear solve (validated to ~5e-7 rel err against the jax reference):
the (T,B) log-buffer collapses to Ut_i = (D_i + a_i s_i^T) Ut_{i-1} in prob
space; the scalar rearrange flux p satisfies p = c + K p with K = tril(alpha
beta^T, -1) rank-16; solved per 128-chunk with a nilpotent doubling inverse
and cross-chunk 16-dim state with rescaling. O(T*B + NC*L^2) host work on
tiny data (the sequential part is irreducible on any backend).
"""
import numpy as np

T, S, B, A = 2048, 512, 16, 18
PEN = 0.5
NCORES = 8
ROWS = 257          # 2049 rows padded to 8*257 = 2056
NPAD = 8 * ROWS
L, NCHUNK = 128, 16

_prog = None


def _build_program():
    import concourse.bass as bass
    import concourse.tile as tile
    from concourse import bacc, mybir

    nc = bacc.Bacc("TRN2", target_bir_lowering=False, debug=False,
                   num_devices=NCORES)
    # bf16 I/O: host<->device transfer over the axon tunnel dominates wall
    # time; PE matmuls bf16 natively with fp32 PSUM accumulation.
    sT = nc.dram_tensor("sT", [S, ROWS], mybir.dt.bfloat16,
                        kind="ExternalInput")
    W = nc.dram_tensor("W", [S, 336], mybir.dt.bfloat16,
                       kind="ExternalInput")
    out = nc.dram_tensor("logits", [ROWS, 336], mybir.dt.bfloat16,
                         kind="ExternalOutput")

    with tile.TileContext(nc) as tc:
        with tc.tile_pool(name="sb", bufs=1) as pool, \
             tc.tile_pool(name="ps", bufs=2, space="PSUM") as pps:
            # plain 2D DMAs, each staged through one compute op so downstream
            # matmuls wait on a single semaphore (walrus caps sync waits per
            # instruction and a wide DMA fans out over many DGE queues)
            sT_sb = pool.tile([128, 4, ROWS], mybir.dt.bfloat16, tag="sT")
            W_sb = pool.tile([128, 4, 336], mybir.dt.bfloat16, tag="W")
            for k in range(4):
                tr = pool.tile([128, ROWS], mybir.dt.bfloat16, tag=f"sTr{k}")
                nc.gpsimd.dma_start(tr[:], sT[k * 128:(k + 1) * 128, :])
                nc.scalar.copy(sT_sb[:, k, :], tr[:])
                wr = pool.tile([128, 336], mybir.dt.bfloat16, tag=f"Wr{k}")
                nc.gpsimd.dma_start(wr[:], W[k * 128:(k + 1) * 128, :])
                nc.scalar.copy(W_sb[:, k, :], wr[:])
            for m, mlen in ((0, 128), (128, 128), (256, 1)):
                ps = pps.tile([mlen, 336], mybir.dt.float32, tag=f"ps{m}")
                for k in range(4):
                    nc.tensor.matmul(ps[:], sT_sb[:, k, m:m + mlen],
                                     W_sb[:, k, :], start=(k == 0),
                                     stop=(k == 3))
                ot = pool.tile([mlen, 336], mybir.dt.bfloat16, tag=f"ot{m}")
                nc.scalar.copy(ot[:], ps[:])
                nc.gpsimd.dma_start(out[m:m + mlen, :], ot[:])
    nc.compile()
    return nc


def _run_device(s_i, Wcat):
    global _prog
    if _prog is None:
        _prog = _build_program()
    import ml_dtypes
    from concourse.bass_utils import run_bass_kernel_spmd
    bf16 = ml_dtypes.bfloat16
    Wb = np.ascontiguousarray(Wcat.astype(bf16))
    in_maps = []
    for c in range(NCORES):
        r0 = c * ROWS
        nrows = min(ROWS, T + 1 - r0)             # last shard is 250 rows
        shard = np.zeros((S, ROWS), bf16)
        shard[:, :nrows] = s_i[r0:r0 + nrows].astype(bf16).T
        in_maps.append({"sT": shard, "W": Wb})
    res = run_bass_kernel_spmd(_prog, in_maps, core_ids=list(range(NCORES)))
    logits = np.concatenate([res.results[c]["logits"] for c in range(NCORES)],
                            axis=0)[:T + 1]
    return logits


def _host_scan(logits, actions):
    f32 = np.float32
    la = logits[:, :288].reshape(T + 1, B, A)
    lst = logits[:, 288:320].reshape(T + 1, B, 2)
    lsr = logits[:, 320:336]
    act = np.asarray(actions).astype(np.int64)
    # heads (bounded logits: no max-shift needed)
    ea = np.exp(la)
    e = (ea[np.arange(T)[:, None], np.arange(B)[None, :], act[:, None]]
         / ea[:T].sum(-1)).astype(f32)
    delta = (lst[:, :, 0] - lst[:, :, 1]).astype(f32)
    expm = np.exp(-delta)
    ds = (1.0 / (1.0 + expm)).astype(f32)
    ss = (expm * ds).astype(f32)
    ld = (-np.log1p(expm)).astype(f32)
    er = np.exp(lsr)
    at = (np.exp(f32(-PEN)) * er / er.sum(-1, keepdims=True)).astype(f32)

    ld = ld.copy()
    ld[0] = 0.0
    C = np.cumsum(ld[:T], 0, dtype=f32)          # C_i global, i=0..T-1
    tril = np.tril(np.ones((L, L), f32), -1)
    tot = 0.0
    logscale = 0.0
    lam_sum = 0.0
    zrow = None
    aux = []
    for c in range(NCHUNK):
        i0 = c * L
        Cl = C[i0:i0 + L]
        Cstart = C[i0 - 1] if c > 0 else np.zeros(B, f32)
        Cm = (0.5 * (Cstart + Cl[-1])).astype(f32)
        Clprev = np.vstack([Cstart, Cl[:-1]])
        alpha = ss[i0:i0 + L] * np.exp(Clprev - Cm)
        beta = at[i0:i0 + L] * np.exp(Cm - Cl)
        if c == 0:
            alpha[0] = 0.0
            beta[0] = 0.0
        K = np.where(tril > 0, alpha @ beta.T, f32(0))
        SA = alpha.copy()
        Ks = K
        for s in range(7):                        # exact: K^0..K^127
            SA = SA + Ks @ SA
            if s < 6:
                Ks = Ks @ Ks
        aux.append((Cl, Cm, beta, SA))
    for c in range(NCHUNK):
        i0 = c * L
        Cl, Cm, beta, SA = aux[c]
        if c == 0:
            zhat = (np.exp(lsr[0]) / np.exp(lsr[0]).sum()
                    * np.exp(Cm)).astype(f32)
        p = SA @ zhat
        Y = zhat[None, :] + np.cumsum(beta * p[:, None], 0, dtype=f32)
        w = ((e[i0:i0 + L] * np.exp(Cl - Cm)) * Y).sum(1)
        tot += np.log(w).sum() + L * logscale
        zend = np.exp(Cl[-1] - Cm) * Y[-1]
        if c < NCHUNK - 1:
            mu = zend.sum()
            zhat = ((zend / mu) * np.exp(aux[c + 1][1] - Cl[-1])).astype(f32)
            logscale += np.log(mu)
    tot += np.log((ds[T] * zend).sum()) + logscale
    return np.float32(tot)


def kernel(s_i, W_action, W_stop, W_start, actions):
    s_i = np.asarray(s_i, np.float32)
    Wcat = np.ascontiguousarray(
        np.concatenate([np.asarray(W_action, np.float32),
                        np.asarray(W_stop, np.float32),
                        np.asarray(W_start, np.float32)], axis=1))
    logits = _run_device(s_i, Wcat)
    return _host_scan(logits.astype(np.float32), actions)
